# revision 22
# baseline (speedup 1.0000x reference)
"""Trainium2 Bass kernel for nn_ContinuousActor (GNN message passing actor MLP).

Strategy (pure data parallel over 8 cores, batch dim sharded):
  - Host repacks inputs feature-major: XT[74, B] = [obs.T; ag.T; g.T; ones].
    The ones row folds the per-pair phi1 bias (incl. one-hot rows) into the
    matmul, so the phi1 psum drain is a pure relu.
  - Per-pair effective phi1 weights W1e[p] [74, 256] built host-side.
  - Device pipeline per 512-col batch tile (feature-major [feat, batch]):
    6x (phi1 -> relu -> phi2 -> fused relu+accumulate) -> rho -> heads+clip.
    Software-pipelined one pair deep: pair p+1's phi1 matmuls are emitted
    before pair p's phi2-m0, and phi2-m1 trails one more block, giving the
    psum drains a ~1.7us window so the PE never waits on a drain round trip.
  - Bias handling: phi2-m0 half via ACT relu+bias; phi2-m1 half via DVE
    scalar_tensor_tensor (max(x,-b2) add acc) with the missing 6*b2_m1
    correction folded into the rho bias (asymmetric fold).
  - Pooling accumulates in bf16: m0 via GPSIMD adds, m1 fused in the STT op.
  - Output stored bf16 [8, bc]; host transposes/upcasts.
"""

import numpy as np
import ml_dtypes
from collections import deque
from contextlib import ExitStack

import concourse.bass as bass
import concourse.mybir as mybir
import concourse.tile as tile
from concourse import bacc
from concourse.bass_utils import run_bass_kernel_spmd

F32 = mybir.dt.float32
BF16 = mybir.dt.bfloat16
RELU = mybir.ActivationFunctionType.Relu
NPBF = ml_dtypes.bfloat16

B_FULL = 65536
N_CORES = 8
BC = B_FULL // N_CORES  # 8192 batch rows per core
BT = 512                # batch tile (matmul free dim)
KX = 74                 # 55 obs + 9 ag + 9 g + 1 ones
PERMS = [(0, 1), (0, 2), (1, 0), (1, 2), (2, 0), (2, 1)]
LOG_SIG_MIN, LOG_SIG_MAX = -20.0, 2.0

_CACHE = {}


def _pack_weights(phi_w1, phi_b1, phi_w2, phi_b2, rho_w1, rho_b1,
                  mean_w, mean_b, logstd_w, logstd_b):
    """Host-side weight repacking into device layouts."""
    f = np.float32
    phi_w1, phi_b1 = np.asarray(phi_w1, f), np.asarray(phi_b1, f)
    phi_w2, phi_b2 = np.asarray(phi_w2, f), np.asarray(phi_b2, f)
    rho_w1, rho_b1 = np.asarray(rho_w1, f), np.asarray(rho_b1, f)
    # w1e: per-pair effective weights [74, 6*256]; ones-row (73) carries bias.
    w1 = np.zeros((KX, 6 * 256), dtype=f)
    for p, (i, j) in enumerate(PERMS):
        Wp = w1[:, p * 256:(p + 1) * 256]
        Wp[0:10] = phi_w1[12:22]                      # obs body
        Wp[10 + 15 * i:25 + 15 * i] = phi_w1[25:40]   # obj i features
        Wp[10 + 15 * j:25 + 15 * j] = phi_w1[43:58]   # obj j features
        Wp[55 + 3 * i:58 + 3 * i] = phi_w1[0:3]       # ag_i
        Wp[55 + 3 * j:58 + 3 * j] = phi_w1[3:6]       # ag_j
        Wp[64 + 3 * i:67 + 3 * i] = phi_w1[6:9]       # g_i
        Wp[64 + 3 * j:67 + 3 * j] = phi_w1[9:12]      # g_j
        Wp[73] = phi_b1 + phi_w1[22 + i] + phi_w1[40 + j]  # bias + one-hots

    # w2/wr: [128, 512] with col block (2k+m) = W[k*128:(k+1)*128, m*128:(m+1)*128]
    def pack_256(w):
        out = np.empty((128, 512), dtype=f)
        for k in range(2):
            for m in range(2):
                out[:, (2 * k + m) * 128:(2 * k + m + 1) * 128] = \
                    w[k * 128:(k + 1) * 128, m * 128:(m + 1) * 128]
        return out
    # wbig = [w2p | wrp | wh] : [128, 512+512+16]
    wh_full = np.concatenate([np.asarray(mean_w, f), np.asarray(logstd_w, f)], axis=1)  # [256, 8]
    wh = np.concatenate([wh_full[0:128, :], wh_full[128:256, :]], axis=1)  # [128, 16]
    wbig = np.concatenate([pack_256(phi_w2), pack_256(rho_w1), wh], axis=1)  # [128, 1040]
    bh = np.concatenate([np.asarray(mean_b, f), np.asarray(logstd_b, f)])  # [8]
    # cpack [128, 7] f32:
    #  c0 = -b2_m1 (STT max threshold), c1 = b2_m0 (ACT bias),
    #  c2/c3 = br_eff m0/m1, c4/c5 = head clip hi/lo SHIFTED by -bh
    #  (clip(x+bh, lo, hi) = clip(x, lo-bh, hi-bh) + bh), c6 = bh
    br_eff = rho_b1 + 6.0 * (phi_b2[128:256] @ rho_w1[128:256, :])  # [256]
    big = np.float32(3.0e38)
    cpack = np.zeros((128, 7), dtype=f)
    cpack[:, 0] = -phi_b2[128:256]
    cpack[:, 1] = phi_b2[0:128]
    cpack[:, 2] = br_eff[0:128]
    cpack[:, 3] = br_eff[128:256]
    cpack[0:4, 4], cpack[4:8, 4] = big, LOG_SIG_MAX - bh[4:8]   # hi (min)
    cpack[0:4, 5], cpack[4:8, 5] = -big, LOG_SIG_MIN - bh[4:8]  # lo (max)
    cpack[0:8, 6] = bh
    return dict(w1=w1.astype(NPBF), wbig=wbig.astype(NPBF), cpack=cpack)


def _pack_xt(obs, ag, g):
    B = obs.shape[0]
    xt = np.empty((KX, B), dtype=NPBF)
    xt[0:55] = obs.T.astype(NPBF)
    xt[55:64] = ag.T.astype(NPBF)
    xt[64:73] = g.T.astype(NPBF)
    xt[73] = np.asarray(1.0, NPBF)
    return xt


def _build_bass(bc, bt):
    nt = bc // bt
    nq = nt * 6  # global pair count
    nc = bacc.Bacc(trn_type="TRN2")

    xt_d = nc.dram_tensor("xt", [KX, bc], BF16, kind="ExternalInput")
    w1_d = nc.dram_tensor("w1", [KX, 6 * 256], BF16, kind="ExternalInput")
    wbig_d = nc.dram_tensor("wbig", [128, 1040], BF16, kind="ExternalInput")
    cpack_d = nc.dram_tensor("cpack", [128, 7], F32, kind="ExternalInput")
    y_d = nc.dram_tensor("y", [8, bc], BF16, kind="ExternalOutput")

    AMIN, AMAX, AADD = mybir.AluOpType.min, mybir.AluOpType.max, mybir.AluOpType.add

    with ExitStack() as ctx:
        tc = ctx.enter_context(tile.TileContext(nc))
        consts = ctx.enter_context(tc.tile_pool(name="consts", bufs=1))
        sbp = ctx.enter_context(tc.tile_pool(name="sbp", bufs=2))
        psp = ctx.enter_context(tc.tile_pool(name="psp", bufs=2, space="PSUM"))

        # --- const loads: xts(0) first so compute starts ASAP -------------
        xts_tiles = {}

        def load_xts(t):
            xts = sbp.tile([KX, bt], BF16, tag="xts", name="xts", bufs=3)
            nc.sync.dma_start(out=xts, in_=xt_d[:, t * bt:(t + 1) * bt])
            xts_tiles[t] = xts

        # Critical path: first matmuls need only xts(0) + w1 pair 0; stage
        # the rest across both DMA queues in need-order.
        load_xts(0)
        w1sb = consts.tile([KX, 6 * 256], BF16)
        nc.sync.dma_start(out=w1sb[:, 0:256], in_=w1_d[:, 0:256])
        cpsb = consts.tile([128, 7], F32)
        nc.scalar.dma_start(out=cpsb, in_=cpack_d[:, :])
        nc.scalar.dma_start(out=w1sb[:, 256:768], in_=w1_d[:, 256:768])
        nc.sync.dma_start(out=w1sb[:, 768:1536], in_=w1_d[:, 768:1536])
        wbsb = consts.tile([128, 1040], BF16)
        nc.scalar.dma_start(out=wbsb[:, 0:512], in_=wbig_d[:, 0:512])
        nc.scalar.dma_start(out=wbsb[:, 512:1040], in_=wbig_d[:, 512:1040])
        load_xts(1)
        load_xts(2)

        w2sb = wbsb[:, 0:512]
        wrsb = wbsb[:, 512:1024]
        whsb = wbsb[:, 1024:1040]

        # HAM warmup: the PE idles ~4.5us waiting for the first DMAs --
        # exactly the span the HAM activity monitor needs to unthrottle the
        # clock gate (1.2 -> 2.4 GHz). Burn it on dummy matmuls over a
        # scratch buffer so the real matmuls start warm.
        dumw = consts.tile([128, bt], BF16)
        nc.gpsimd.memset(dumw, 0.0)
        dups = psp.tile([128, bt], F32, tag="pha", name="dups")
        for _ in range(10):
            nc.tensor.matmul(dups, dumw[:, 0:128], dumw, start=True, stop=True)

        # --- per-pair state ----------------------------------------------
        ph1s, h1s, accs = {}, {}, {}
        fin_q = deque()

        def stage1(q):
            """phi1 matmuls for global pair q + psum drain (pure relu)."""
            t, p = divmod(q, 6)
            if p == 0:
                if t + 3 < nt:
                    load_xts(t + 3)
                accs[t] = sbp.tile([128, 2 * bt], BF16, tag="acc", name="acc")
            xts = xts_tiles[t]
            ph1 = psp.tile([128, 2 * bt], F32, tag="ph1", name="ph1")
            for m in range(2):
                nc.tensor.matmul(
                    ph1[:, m * bt:(m + 1) * bt],
                    w1sb[:, p * 256 + m * 128:p * 256 + (m + 1) * 128],
                    xts, start=True, stop=True,
                )
            h1 = sbp.tile([128, 2 * bt], BF16, tag="h1", name="h1", bufs=4)
            if (p + t) % 2 == 1:
                nc.vector.tensor_scalar_max(h1, ph1, 0.0)
            else:
                nc.scalar.activation(h1, ph1, RELU)
            ph1s[q], h1s[q] = ph1, h1

        def stage2(q):
            """phi2 m0 matmuls for pair q + ACT relu+bias consumer."""
            t, p = divmod(q, 6)
            h1, acc = h1s[q], accs[t]
            pha = psp.tile([128, bt], F32, tag="pha", name="pha")
            for k in range(2):
                nc.tensor.matmul(
                    pha, w2sb[:, (2 * k) * 128:(2 * k + 1) * 128],
                    h1[:, k * bt:(k + 1) * bt], start=(k == 0), stop=(k == 1),
                )
            if p == 0:
                nc.scalar.activation(acc[:, 0:bt], pha, RELU, bias=cpsb[:, 1:2])
            else:
                rm0 = sbp.tile([128, bt], BF16, tag="rm0", name="rm0", bufs=3)
                nc.scalar.activation(rm0, pha, RELU, bias=cpsb[:, 1:2])
                nc.gpsimd.tensor_add(acc[:, 0:bt], acc[:, 0:bt], rm0)

        def stage3(q):
            """phi2 m1 matmuls for pair q + DVE fused relu/accumulate."""
            t, p = divmod(q, 6)
            h1, acc = h1s[q], accs[t]
            phb = psp.tile([128, bt], F32, tag="phb", name="phb")
            for k in range(2):
                nc.tensor.matmul(
                    phb, w2sb[:, (2 * k + 1) * 128:(2 * k + 2) * 128],
                    h1[:, k * bt:(k + 1) * bt], start=(k == 0), stop=(k == 1),
                )
            if p == 0:
                nc.vector.tensor_scalar(
                    acc[:, bt:2 * bt], phb, cpsb[:, 0:1], 0.0,
                    op0=AMAX, op1=AADD,
                )
            else:
                nc.vector.scalar_tensor_tensor(
                    acc[:, bt:2 * bt], phb, cpsb[:, 0:1], acc[:, bt:2 * bt],
                    op0=AMAX, op1=AADD,
                )
            del ph1s[q], h1s[q]

        def finisher(t):
            """rho + heads + clip + store for tile t, as 3 weavable stages."""
            acc = accs[t]
            st = {}

            def stage_a():  # rho m0
                pr0 = psp.tile([128, bt], F32, tag="phb", name="pr0", bufs=2)
                for k in range(2):
                    nc.tensor.matmul(
                        pr0, wrsb[:, (2 * k) * 128:(2 * k + 1) * 128],
                        acc[:, k * bt:(k + 1) * bt],
                        start=(k == 0), stop=(k == 1),
                    )
                xs = sbp.tile([128, 2 * bt], BF16, tag="xs", name="xs")
                nc.scalar.activation(xs[:, 0:bt], pr0, RELU, bias=cpsb[:, 2:3])
                st["xs"] = xs

            def stage_b():  # rho m1
                pr1 = psp.tile([128, bt], F32, tag="phb", name="pr1", bufs=2)
                for k in range(2):
                    nc.tensor.matmul(
                        pr1, wrsb[:, (2 * k + 1) * 128:(2 * k + 2) * 128],
                        acc[:, k * bt:(k + 1) * bt],
                        start=(k == 0), stop=(k == 1),
                    )
                nc.scalar.activation(st["xs"][:, bt:2 * bt], pr1, RELU,
                                     bias=cpsb[:, 3:4])

            def stage_c():  # heads + clip(+bias via shifted bounds) + store
                xs = st["xs"]
                py = psp.tile([8, bt], F32, tag="pha", name="py", bufs=2)
                for k in range(2):
                    nc.tensor.matmul(py, whsb[:, k * 8:(k + 1) * 8],
                                     xs[:, k * bt:(k + 1) * bt],
                                     start=(k == 0), stop=(k == 1))
                yt = sbp.tile([8, bt], BF16, tag="yt", name="yt")
                nc.vector.tensor_scalar(
                    yt, py, cpsb[0:8, 4:5], cpsb[0:8, 5:6],
                    op0=AMIN, op1=AMAX,
                )
                ysb = sbp.tile([8, bt], BF16, tag="ysb", name="ysb")
                nc.vector.tensor_scalar_add(ysb, yt, cpsb[0:8, 6:7])
                nc.sync.dma_start(out=y_d[:, t * bt:(t + 1) * bt], in_=ysb)

            return [stage_a, stage_b, stage_c]

        # --- master emission loop (1-pair software skew) ------------------
        for q in range(nq + 2):
            t, p = divmod(q, 6)
            if q < nq:
                stage1(q)
            if 0 <= q - 2 < nq:
                stage2(q - 2)
                stage3(q - 2)
            if p in (1, 3, 5) and fin_q:
                fin_q.popleft()()
            if q - 2 >= 0 and (q - 2) % 6 == 5:
                fin_q.extend(finisher((q - 2) // 6))
        while fin_q:
            fin_q.popleft()()

    return nc


def _get_nc(bc, bt):
    key = (bc, bt)
    if key not in _CACHE:
        nc = _build_bass(bc, bt)
        nc.finalize()
        _CACHE[key] = nc
    return _CACHE[key]


def kernel(obs, ag, g, phi_w1, phi_b1, phi_w2, phi_b2,
           rho_w1, rho_b1, mean_w, mean_b, logstd_w, logstd_b):
    obs = np.asarray(obs, np.float32)
    ag = np.asarray(ag, np.float32)
    g = np.asarray(g, np.float32)
    B = obs.shape[0]
    assert B == B_FULL, f"kernel hardcoded for B={B_FULL}, got {B}"

    packed = _pack_weights(phi_w1, phi_b1, phi_w2, phi_b2, rho_w1, rho_b1,
                           mean_w, mean_b, logstd_w, logstd_b)
    xt = _pack_xt(obs, ag, g)

    nc = _get_nc(BC, BT)
    in_maps = []
    for c in range(N_CORES):
        m = dict(packed)
        m["xt"] = np.ascontiguousarray(xt[:, c * BC:(c + 1) * BC])
        in_maps.append(m)

    import os
    trace = bool(os.environ.get("KERNEL_TRACE"))
    res = run_bass_kernel_spmd(nc, in_maps, core_ids=list(range(N_CORES)),
                               trace=trace)
    global _last_results
    _last_results = res

    y = np.concatenate(
        [np.asarray(res.results[c]["y"]) for c in range(N_CORES)], axis=1)
    out = np.ascontiguousarray(y.T.astype(np.float32))  # [B, 8]
    mean = out[:, 0:4].copy()
    logstd = out[:, 4:8].copy()
    return mean, logstd


_last_results = None
